# revision 13
# baseline (speedup 1.0000x reference)
"""GCN layer (GCNConv + log_softmax) on 8 Trainium2 NeuronCores.

Sharding: nodes row-sharded 8 ways. Each core computes h' = dis_src * (x @ W)
for its slice (bf16 matmuls, fp8-e3m4 stored), AllGathers h' in two chunks
(A: first 3584 local rows, fired at 4/7 of the GEMM; B: last 2688), then
aggregates edge messages for its destination slice with batched dma_gather
calls (7 dst-tiles per call to amortize the ~6us fixed Q7 descriptor-gen
cost) + pure-0/1 one-hot matmuls on the tensor engine.  dst normalization
(dis_dst) and bias are applied in the epilogue; log_softmax skips the
max-subtraction (|z| < 5 for this distribution, exp stays in fp32 range).
Self-loop messages never enter the gather stream: they are folded into the
per-tile partial accumulator during the GEMM phase.
"""

import numpy as np
import ml_dtypes

import concourse.bass as bass
import concourse.tile as tile
from concourse import bacc, mybir
from concourse.bass_utils import run_bass_kernel_spmd

bf16 = ml_dtypes.bfloat16
e3m4 = ml_dtypes.float8_e3m4
F32 = mybir.dt.float32
BF16 = mybir.dt.bfloat16
I16 = mybir.dt.int16
import os
USE_FP8 = os.environ.get("GCN_FP8", "1") == "1"
MSG_NP = e3m4 if USE_FP8 else bf16
MSG_DT = mybir.dt.float8e3 if USE_FP8 else BF16

N_NODES = 50000
D_IN = 2048
D_OUT = 512
C = 8                      # cores
NLOC = N_NODES // C        # 6250 real nodes per core
T = 49                     # dst tiles per core
NPAD = T * 128             # 6272 padded rows per core
SUP = 7                    # GEMM row-chunks per core
SW = NPAD // SUP           # 896 rows per chunk
SUP_A = 4                  # chunks feeding AllGather A (fires early)
LOC_A = SUP_A * SW         # 3584 local rows in A
LOC_B = NPAD - LOC_A       # 2688 local rows in B
ROWS_A = C * LOC_A         # 28672 rows in h_A  (< 32768 -> int16 ok)
ROWS_B = C * LOC_B         # 21504 rows in h_B
KT = D_IN // 128           # 16 contraction chunks
G = int(os.environ.get("GCN_G", "0")) or (7 if USE_FP8 else 4)
NG = -(-T // G)            # gather groups (last may be short)
USE_IDMM = os.environ.get("GCN_IDMM", "1") == "1"

LAST_RESULTS = None        # test harness reads exec_time_ns from here


def _patch_act_tables():
    """Prefer the combined ln+exp act-function table so the per-tile epilogue
    (Exp then Ln) does not reload the piecewise-poly table 2x per tile."""
    import functools
    from concourse import hw_specs
    orig = hw_specs.get_activation_tables
    if getattr(hw_specs, "_gcn_act_patch", False):
        return
    base = orig.__wrapped__ if hasattr(orig, "__wrapped__") else orig

    @functools.cache
    def patched(module_arch):
        tabs = base(module_arch)
        pref = [k for k in tabs if k == "natural_log_exp_and_others"]
        if pref:
            order = pref + [k for k in tabs if k not in pref]
            tabs = {k: tabs[k] for k in order}
        return tabs

    hw_specs.get_activation_tables = patched
    bacc.get_activation_tables = patched
    hw_specs._gcn_act_patch = True


def _wrap_idx(idx):
    """Wrap a [n] index array into the [128, n//16] dma_gather layout."""
    n = idx.shape[0]
    assert n % 16 == 0
    cols = n // 16
    w = np.empty((128, cols), np.int16)
    blk = idx.reshape(cols, 16).T.astype(np.int16)   # [16, cols]
    for g in range(8):
        w[g * 16:(g + 1) * 16, :] = blk
    return w


def _preprocess(x, edge_index, weight, bias):
    src = np.asarray(edge_index[0], dtype=np.int64)
    dst = np.asarray(edge_index[1], dtype=np.int64)
    loops = np.arange(N_NODES, dtype=np.int64)

    deg = np.bincount(np.concatenate([dst, loops]), minlength=N_NODES)
    dis = 1.0 / np.sqrt(deg.astype(np.float32))   # deg >= 1 via self loops

    # message stream excludes self loops (folded into partial during GEMM)
    sc = src // NLOC
    sr = src % NLOC
    half = (sr >= LOC_A).astype(np.int64)
    g = np.where(half == 0, sc * LOC_A + sr, sc * LOC_B + (sr - LOC_A))

    dc = dst // NLOC                   # dst core
    dr = dst % NLOC
    dt = dr // 128                     # dst tile within core
    dl = dr % 128                      # dst row within tile

    order = np.lexsort((g, half, dt, dc))
    g, dc, dt, dl, half = g[order], dc[order], dt[order], dl[order], half[order]

    key = (dc * T + dt) * 2 + half
    counts = np.bincount(key, minlength=C * T * 2).reshape(C, T, 2)
    blocks = -(-counts // 128)                       # ceil div
    B_A = blocks[:, :, 0].max(axis=0)                # [T]
    B_B = blocks[:, :, 1].max(axis=0)                # [T]

    GA = np.array([B_A[gi * G:min((gi + 1) * G, T)].sum() for gi in range(NG)])
    GB = np.array([B_B[gi * G:min((gi + 1) * G, T)].sum() for gi in range(NG)])
    blk_cols = int(GA.sum() + GB.sum())
    idx_cols = 8 * blk_cols
    idx_np = np.zeros((C, 128, idx_cols), np.int16)
    oh_np = np.zeros((C, 128, blk_cols * 128), MSG_NP)

    starts = np.zeros(C * T * 2 + 1, np.int64)
    np.cumsum(np.bincount(key, minlength=C * T * 2), out=starts[1:])

    dcol = np.arange(128)
    for c in range(C):
        icol = 0
        bcol = 0
        for h in (0, 1):
            B_h = B_A if h == 0 else B_B
            for gi in range(NG):
                gidx = []
                for t in range(gi * G, min((gi + 1) * G, T)):
                    B = int(B_h[t])
                    k = (c * T + t) * 2 + h
                    seg = slice(starts[k], starts[k + 1])
                    n = int(starts[k + 1] - starts[k])
                    cap = B * 128
                    gvec = np.zeros(cap, np.int64)
                    gvec[:n] = g[seg]
                    gidx.append(gvec)
                    dv = np.full(cap, -1.0, np.float32)
                    dv[:n] = dl[seg]
                    ohb = (dv.reshape(B, 128)[:, :, None] == dcol[None, None, :])
                    oh_np[c, :, bcol * 128:(bcol + B) * 128] = (
                        ohb.transpose(1, 0, 2).reshape(128, B * 128).astype(MSG_NP))
                    bcol += B
                gidx = np.concatenate(gidx)
                idx_np[c, :, icol:icol + 8 * (len(gidx) // 128)] = _wrap_idx(gidx)
                icol += 8 * (len(gidx) // 128)

    w_bf = np.ascontiguousarray(weight.astype(bf16))
    xT = np.zeros((C, D_IN, NPAD), bf16)
    dis_np = np.zeros((C, 128, T), np.float32)
    for c in range(C):
        xs = x[c * NLOC:(c + 1) * NLOC]
        xT[c, :, :NLOC] = xs.T.astype(bf16)
        dis_np[c, :, :] = np.pad(dis[c * NLOC:(c + 1) * NLOC],
                                 (0, NPAD - NLOC)).reshape(T, 128).T

    bias_full = np.tile(np.asarray(bias, np.float32)[None, :], (128, 1))
    ident = np.eye(128, dtype=bf16)

    return dict(
        B_A=B_A, B_B=B_B, GA=GA, GB=GB, idx=idx_np, oh=oh_np, w=w_bf, xT=xT,
        dis=dis_np, bias=np.ascontiguousarray(bias_full), ident=ident,
        bias_zero=not np.any(np.asarray(bias)),
    )


def _build(B_A, B_B, GA, GB, idx_cols, blk_cols, bias_zero):
    nc = bacc.Bacc("TRN2", target_bir_lowering=False, debug=False,
                   num_devices=C)

    xT_t = nc.dram_tensor("xT", [D_IN, NPAD], BF16, kind="ExternalInput")
    w_t = nc.dram_tensor("w", [D_IN, D_OUT], BF16, kind="ExternalInput")
    dis_t = nc.dram_tensor("dis", [128, T], F32, kind="ExternalInput")
    bias_t = nc.dram_tensor("biasf", [128, D_OUT], F32, kind="ExternalInput")
    idx_t = nc.dram_tensor("idx", [128, idx_cols], I16, kind="ExternalInput")
    oh_t = nc.dram_tensor("oh", [128, blk_cols * 128], MSG_DT,
                          kind="ExternalInput")
    id_t = nc.dram_tensor("ident", [128, 128], BF16, kind="ExternalInput")
    out_t = nc.dram_tensor("out", [NPAD, D_OUT], BF16, kind="ExternalOutput")

    xT, w, dis, biasf, idx, oh, ident, out = (
        t.ap() for t in (xT_t, w_t, dis_t, bias_t, idx_t, oh_t, id_t, out_t))

    # column offsets: idx / oh blocks laid out A-groups then B-groups
    gcolA = np.zeros(NG, np.int64)     # idx col offset of A-group g
    gcolB = np.zeros(NG, np.int64)
    bcol_t = np.zeros((T, 2), np.int64)  # oh block offset of (tile, half)
    ic = bc = 0
    for gi in range(NG):
        gcolA[gi] = ic
        for t in range(gi * G, min((gi + 1) * G, T)):
            bcol_t[t, 0] = bc
            bc += int(B_A[t])
        ic += 8 * int(GA[gi])
    for gi in range(NG):
        gcolB[gi] = ic
        for t in range(gi * G, min((gi + 1) * G, T)):
            bcol_t[t, 1] = bc
            bc += int(B_B[t])
        ic += 8 * int(GB[gi])

    maxGA = int(max(GA.max(), GB.max()))

    with tile.TileContext(nc) as tc:
        with tc.tile_pool(name="const", bufs=1) as constp, \
             tc.tile_pool(name="xk", bufs=2) as xkp, \
             tc.tile_pool(name="hq", bufs=3) as hqp, \
             tc.tile_pool(name="gath", bufs=2) as gp, \
             tc.tile_pool(name="ohp", bufs=2) as ohp, \
             tc.tile_pool(name="epi", bufs=3) as epip, \
             tc.tile_pool(name="psum", bufs=4, space="PSUM") as psp, \
             tc.tile_pool(name="dram", bufs=1, space="DRAM") as dramp:

            # resident constants
            w_sb = constp.tile([128, KT, D_OUT], BF16)
            for k in range(KT):
                nc.sync.dma_start(out=w_sb[:, k, :], in_=w[k * 128:(k + 1) * 128, :])
            dis_sb = constp.tile([128, T], F32)
            nc.sync.dma_start(out=dis_sb[:], in_=dis[:])
            bias_sb = constp.tile([128, D_OUT], F32)
            nc.sync.dma_start(out=bias_sb[:], in_=biasf[:])
            idx_sb = constp.tile([128, idx_cols], I16)
            nc.sync.dma_start(out=idx_sb[:], in_=idx[:])
            ident_sb = constp.tile([128, 128], BF16)
            nc.sync.dma_start(out=ident_sb[:], in_=ident[:])
            partial = constp.tile([128, T, D_OUT], BF16)
            smbuf = constp.tile([128, T], F32)

            h_locA = dramp.tile([LOC_A, D_OUT], MSG_DT)
            h_locB = dramp.tile([LOC_B, D_OUT], MSG_DT)
            h_A = dramp.tile([ROWS_A, D_OUT], MSG_DT, addr_space="Shared")
            h_B = dramp.tile([ROWS_B, D_OUT], MSG_DT, addr_space="Shared")

            # ---- phase 1: h' = dis_src * (x @ W); partial init; 2 AGs ----
            for s in range(SUP):
                xk = xkp.tile([128, KT, SW], BF16, name="xk")
                for k in range(KT):
                    nc.sync.dma_start(
                        out=xk[:, k, :],
                        in_=xT[k * 128:(k + 1) * 128, s * SW:(s + 1) * SW])
                for t in range(SW // 128):
                    ph = psp.tile([128, D_OUT], F32, name="ph")
                    for k in range(KT):
                        nc.tensor.matmul(
                            ph[:], xk[:, k, t * 128:(t + 1) * 128],
                            w_sb[:, k, :], start=(k == 0), stop=(k == KT - 1))
                    gt = s * (SW // 128) + t
                    # partial <- h' tile (self-loop message, bf16)
                    nc.vector.tensor_scalar(
                        partial[:, gt, :], ph[:], dis_sb[:, gt:gt + 1], None,
                        mybir.AluOpType.mult)
                    if not bias_zero:
                        inv = epip.tile([128, 1], F32, name="inv", tag="inv")
                        nc.vector.reciprocal(inv[:], dis_sb[:, gt:gt + 1])
                        tmp = epip.tile([128, D_OUT], F32, name="tmp", tag="tmp")
                        nc.vector.tensor_scalar(
                            tmp[:], bias_sb[:], inv[:, 0:1], None,
                            mybir.AluOpType.mult)
                        nc.vector.tensor_tensor(
                            partial[:, gt, :], partial[:, gt, :], tmp[:],
                            mybir.AluOpType.add)
                    # fp8 copy for the allgather (scalar engine, idle here)
                    hq = hqp.tile([128, D_OUT], MSG_DT, name="hq")
                    nc.scalar.activation(hq[:], partial[:, gt, :],
                                         mybir.ActivationFunctionType.Copy)
                    r0 = gt * 128
                    if r0 < LOC_A:
                        nc.sync.dma_start(out=h_locA[r0:r0 + 128, :], in_=hq[:])
                    else:
                        nc.sync.dma_start(
                            out=h_locB[r0 - LOC_A:r0 - LOC_A + 128, :], in_=hq[:])
                if s == SUP_A - 1:
                    nc.gpsimd.collective_compute(
                        "AllGather", mybir.AluOpType.bypass,
                        replica_groups=[list(range(C))],
                        ins=[h_locA.opt()], outs=[h_A.opt()])
            nc.gpsimd.collective_compute(
                "AllGather", mybir.AluOpType.bypass,
                replica_groups=[list(range(C))],
                ins=[h_locB.opt()], outs=[h_B.opt()])

            # ---- phase 2a: A-half gathers + matmuls accumulate into partial
            for gi in range(NG):
                nga = int(GA[gi])
                ga = gp.tile([128, maxGA, D_OUT], MSG_DT, name="ga")
                icol = int(gcolA[gi])
                nc.gpsimd.dma_gather(
                    out_ap=ga[:, :nga, :], in_ap=h_A[:],
                    idxs_ap=idx_sb[:, icol:icol + 8 * nga],
                    num_idxs=nga * 128, num_idxs_reg=nga * 128,
                    elem_size=D_OUT, single_packet=(nga * 128 <= 1024))
                oh_g = ohp.tile([128, maxGA * 128], MSG_DT, name="oh_g")
                b0 = int(bcol_t[gi * G, 0])
                nc.sync.dma_start(out=oh_g[:, :nga * 128],
                                  in_=oh[:, b0 * 128:(b0 + nga) * 128])
                boff = 0
                for t in range(gi * G, min((gi + 1) * G, T)):
                    ba = int(B_A[t])
                    if ba == 0:
                        continue
                    pa = psp.tile([128, D_OUT], F32, name="pa", tag="ph")
                    for b in range(ba):
                        nc.tensor.matmul(
                            pa[:], oh_g[:, (boff + b) * 128:(boff + b + 1) * 128],
                            ga[:, boff + b, :], start=(b == 0), stop=(b == ba - 1))
                    boff += ba
                    nc.vector.tensor_tensor(partial[:, t, :], partial[:, t, :],
                                            pa[:], mybir.AluOpType.add)

            # ---- phase 2b: B-half gathers + matmuls + epilogue ----
            for gi in range(NG):
                ngb = int(GB[gi])
                gb = gp.tile([128, maxGA, D_OUT], MSG_DT, name="gb", tag="ga")
                icol = int(gcolB[gi])
                nc.gpsimd.dma_gather(
                    out_ap=gb[:, :ngb, :], in_ap=h_B[:],
                    idxs_ap=idx_sb[:, icol:icol + 8 * ngb],
                    num_idxs=ngb * 128, num_idxs_reg=ngb * 128,
                    elem_size=D_OUT, single_packet=(ngb * 128 <= 1024))
                oh_g = ohp.tile([128, maxGA * 128], MSG_DT, name="oh_gb", tag="oh_g")
                b0 = int(bcol_t[gi * G, 1])
                nc.sync.dma_start(out=oh_g[:, :ngb * 128],
                                  in_=oh[:, b0 * 128:(b0 + ngb) * 128])
                boff = 0
                for t in range(gi * G, min((gi + 1) * G, T)):
                    bb = int(B_B[t])
                    acc = psp.tile([128, D_OUT], F32, name="acc")
                    if USE_IDMM:
                        # load partial into PSUM via identity matmul, then
                        # accumulate the B-half one-hot matmuls on top
                        nc.tensor.matmul(acc[:], ident_sb[:], partial[:, t, :],
                                         start=True, stop=(bb == 0))
                        for b in range(bb):
                            nc.tensor.matmul(
                                acc[:],
                                oh_g[:, (boff + b) * 128:(boff + b + 1) * 128],
                                gb[:, boff + b, :], start=False,
                                stop=(b == bb - 1))
                    else:
                        for b in range(bb):
                            nc.tensor.matmul(
                                acc[:],
                                oh_g[:, (boff + b) * 128:(boff + b + 1) * 128],
                                gb[:, boff + b, :], start=(b == 0),
                                stop=(b == bb - 1))
                        nc.vector.tensor_tensor(acc[:], acc[:],
                                                partial[:, t, :],
                                                mybir.AluOpType.add)
                    boff += bb

                    # z = dis_dst * acc (bias folded via partial); overwrite
                    # the dead partial slice with z (bf16)
                    nc.vector.tensor_scalar(partial[:, t, :], acc[:],
                                            dis_sb[:, t:t + 1], None,
                                            mybir.AluOpType.mult)
                    # Exp + per-row sum; Ln batched after the loop (one act
                    # table load instead of two per tile)
                    ex = epip.tile([128, D_OUT], BF16, name="ex")
                    nc.scalar.activation(ex[:], partial[:, t, :],
                                         mybir.ActivationFunctionType.Exp,
                                         accum_out=smbuf[:, t:t + 1])

            # ---- batched log-sum-exp + output sweep ----
            lse = epip.tile([128, T], F32, name="lse", tag="lse")
            nc.scalar.activation(lse[:], smbuf[:],
                                 mybir.ActivationFunctionType.Ln)
            nlse = epip.tile([128, T], F32, name="nlse", tag="nlse")
            nc.vector.tensor_scalar(nlse[:], lse[:], -1.0, None,
                                    mybir.AluOpType.mult)
            for t in range(T):
                res = epip.tile([128, D_OUT], BF16, name="res")
                nc.vector.tensor_scalar(res[:], partial[:, t, :],
                                        nlse[:, t:t + 1], None,
                                        mybir.AluOpType.add)
                nc.sync.dma_start(out=out[t * 128:(t + 1) * 128, :], in_=res[:])

    nc.compile()
    return nc


def kernel(x, edge_index, weight, bias):
    global LAST_RESULTS
    x = np.asarray(x, dtype=np.float32)
    weight = np.asarray(weight, dtype=np.float32)
    bias = np.asarray(bias, dtype=np.float32)

    pp = _preprocess(x, edge_index, weight, bias)
    idx_cols = pp["idx"].shape[2]
    blk_cols = pp["oh"].shape[2] // 128
    nc = _build(pp["B_A"], pp["B_B"], pp["GA"], pp["GB"], idx_cols, blk_cols,
                pp["bias_zero"])

    in_maps = []
    for c in range(C):
        in_maps.append({
            "xT": np.ascontiguousarray(pp["xT"][c]),
            "w": pp["w"],
            "dis": np.ascontiguousarray(pp["dis"][c]),
            "biasf": pp["bias"],
            "idx": np.ascontiguousarray(pp["idx"][c]),
            "oh": np.ascontiguousarray(pp["oh"][c]),
            "ident": pp["ident"],
        })

    res = run_bass_kernel_spmd(nc, in_maps, core_ids=list(range(C)))
    LAST_RESULTS = res

    out = np.empty((N_NODES, D_OUT), np.float32)
    for c in range(C):
        out[c * NLOC:(c + 1) * NLOC] = res.results[c]["out"][:NLOC].astype(np.float32)
    return out


# revision 14
# speedup vs baseline: 1.0429x; 1.0429x over previous
"""GCN layer (GCNConv + log_softmax) on 8 Trainium2 NeuronCores.

Sharding: nodes row-sharded 8 ways. Each core computes h' = dis_src * (x @ W)
for its slice (bf16 matmuls, fp8-e3m4 stored), AllGathers h' in two chunks
(A: first 3584 local rows, fired at 4/7 of the GEMM; B: last 2688), then
aggregates edge messages for its destination slice with batched dma_gather
calls (7 dst-tiles per call to amortize the ~6us fixed Q7 descriptor-gen
cost) + pure-0/1 one-hot matmuls on the tensor engine.  dst normalization
(dis_dst) and bias are applied in the epilogue; log_softmax skips the
max-subtraction (|z| < 5 for this distribution, exp stays in fp32 range).
Self-loop messages never enter the gather stream: they are folded into the
per-tile partial accumulator during the GEMM phase.
"""

import numpy as np
import ml_dtypes

import concourse.bass as bass
import concourse.tile as tile
from concourse import bacc, mybir
from concourse.bass_utils import run_bass_kernel_spmd

bf16 = ml_dtypes.bfloat16
e3m4 = ml_dtypes.float8_e3m4
F32 = mybir.dt.float32
BF16 = mybir.dt.bfloat16
I16 = mybir.dt.int16
import os
USE_FP8 = os.environ.get("GCN_FP8", "1") == "1"
MSG_NP = e3m4 if USE_FP8 else bf16
MSG_DT = mybir.dt.float8e3 if USE_FP8 else BF16

N_NODES = 50000
D_IN = 2048
D_OUT = 512
C = 8                      # cores
NLOC = N_NODES // C        # 6250 real nodes per core
T = 49                     # dst tiles per core
NPAD = T * 128             # 6272 padded rows per core
SUP = 7                    # GEMM row-chunks per core
SW = NPAD // SUP           # 896 rows per chunk
SUP_A = 3                  # chunks feeding AllGather A (fires early)
LOC_A = SUP_A * SW         # 3584 local rows in A
LOC_B = NPAD - LOC_A       # 2688 local rows in B
ROWS_A = C * LOC_A         # 28672 rows in h_A  (< 32768 -> int16 ok)
ROWS_B = C * LOC_B         # 21504 rows in h_B
KT = D_IN // 128           # 16 contraction chunks
G = int(os.environ.get("GCN_G", "0")) or (7 if USE_FP8 else 4)
NG = -(-T // G)            # gather groups (last may be short)
USE_IDMM = os.environ.get("GCN_IDMM", "1") == "1"

LAST_RESULTS = None        # test harness reads exec_time_ns from here


def _patch_act_tables():
    """Prefer the combined ln+exp act-function table so the per-tile epilogue
    (Exp then Ln) does not reload the piecewise-poly table 2x per tile."""
    import functools
    from concourse import hw_specs
    orig = hw_specs.get_activation_tables
    if getattr(hw_specs, "_gcn_act_patch", False):
        return
    base = orig.__wrapped__ if hasattr(orig, "__wrapped__") else orig

    @functools.cache
    def patched(module_arch):
        tabs = base(module_arch)
        pref = [k for k in tabs if k == "natural_log_exp_and_others"]
        if pref:
            order = pref + [k for k in tabs if k not in pref]
            tabs = {k: tabs[k] for k in order}
        return tabs

    hw_specs.get_activation_tables = patched
    bacc.get_activation_tables = patched
    hw_specs._gcn_act_patch = True


def _wrap_idx(idx):
    """Wrap a [n] index array into the [128, n//16] dma_gather layout."""
    n = idx.shape[0]
    assert n % 16 == 0
    cols = n // 16
    w = np.empty((128, cols), np.int16)
    blk = idx.reshape(cols, 16).T.astype(np.int16)   # [16, cols]
    for g in range(8):
        w[g * 16:(g + 1) * 16, :] = blk
    return w


def _preprocess(x, edge_index, weight, bias):
    src = np.asarray(edge_index[0], dtype=np.int64)
    dst = np.asarray(edge_index[1], dtype=np.int64)
    loops = np.arange(N_NODES, dtype=np.int64)

    deg = np.bincount(np.concatenate([dst, loops]), minlength=N_NODES)
    dis = 1.0 / np.sqrt(deg.astype(np.float32))   # deg >= 1 via self loops

    # message stream excludes self loops (folded into partial during GEMM)
    sc = src // NLOC
    sr = src % NLOC
    half = (sr >= LOC_A).astype(np.int64)
    g = np.where(half == 0, sc * LOC_A + sr, sc * LOC_B + (sr - LOC_A))

    dc = dst // NLOC                   # dst core
    dr = dst % NLOC
    dt = dr // 128                     # dst tile within core
    dl = dr % 128                      # dst row within tile

    order = np.lexsort((g, half, dt, dc))
    g, dc, dt, dl, half = g[order], dc[order], dt[order], dl[order], half[order]

    key = (dc * T + dt) * 2 + half
    counts = np.bincount(key, minlength=C * T * 2).reshape(C, T, 2)
    blocks = -(-counts // 128)                       # ceil div
    B_A = blocks[:, :, 0].max(axis=0)                # [T]
    B_B = blocks[:, :, 1].max(axis=0)                # [T]

    GA = np.array([B_A[gi * G:min((gi + 1) * G, T)].sum() for gi in range(NG)])
    GB = np.array([B_B[gi * G:min((gi + 1) * G, T)].sum() for gi in range(NG)])
    blk_cols = int(GA.sum() + GB.sum())
    idx_cols = 8 * blk_cols
    idx_np = np.zeros((C, 128, idx_cols), np.int16)
    oh_np = np.zeros((C, 128, blk_cols * 128), MSG_NP)

    starts = np.zeros(C * T * 2 + 1, np.int64)
    np.cumsum(np.bincount(key, minlength=C * T * 2), out=starts[1:])

    dcol = np.arange(128)
    for c in range(C):
        icol = 0
        bcol = 0
        for h in (0, 1):
            B_h = B_A if h == 0 else B_B
            for gi in range(NG):
                gidx = []
                for t in range(gi * G, min((gi + 1) * G, T)):
                    B = int(B_h[t])
                    k = (c * T + t) * 2 + h
                    seg = slice(starts[k], starts[k + 1])
                    n = int(starts[k + 1] - starts[k])
                    cap = B * 128
                    gvec = np.zeros(cap, np.int64)
                    gvec[:n] = g[seg]
                    gidx.append(gvec)
                    dv = np.full(cap, -1.0, np.float32)
                    dv[:n] = dl[seg]
                    ohb = (dv.reshape(B, 128)[:, :, None] == dcol[None, None, :])
                    oh_np[c, :, bcol * 128:(bcol + B) * 128] = (
                        ohb.transpose(1, 0, 2).reshape(128, B * 128).astype(MSG_NP))
                    bcol += B
                gidx = np.concatenate(gidx)
                idx_np[c, :, icol:icol + 8 * (len(gidx) // 128)] = _wrap_idx(gidx)
                icol += 8 * (len(gidx) // 128)

    w_bf = np.ascontiguousarray(weight.astype(bf16))
    xT = np.zeros((C, D_IN, NPAD), bf16)
    dis_np = np.zeros((C, 128, T), np.float32)
    for c in range(C):
        xs = x[c * NLOC:(c + 1) * NLOC]
        xT[c, :, :NLOC] = xs.T.astype(bf16)
        dis_np[c, :, :] = np.pad(dis[c * NLOC:(c + 1) * NLOC],
                                 (0, NPAD - NLOC)).reshape(T, 128).T

    bias_full = np.tile(np.asarray(bias, np.float32)[None, :], (128, 1))
    ident = np.eye(128, dtype=bf16)

    return dict(
        B_A=B_A, B_B=B_B, GA=GA, GB=GB, idx=idx_np, oh=oh_np, w=w_bf, xT=xT,
        dis=dis_np, bias=np.ascontiguousarray(bias_full), ident=ident,
        bias_zero=not np.any(np.asarray(bias)),
    )


def _build(B_A, B_B, GA, GB, idx_cols, blk_cols, bias_zero):
    nc = bacc.Bacc("TRN2", target_bir_lowering=False, debug=False,
                   num_devices=C)

    xT_t = nc.dram_tensor("xT", [D_IN, NPAD], BF16, kind="ExternalInput")
    w_t = nc.dram_tensor("w", [D_IN, D_OUT], BF16, kind="ExternalInput")
    dis_t = nc.dram_tensor("dis", [128, T], F32, kind="ExternalInput")
    bias_t = nc.dram_tensor("biasf", [128, D_OUT], F32, kind="ExternalInput")
    idx_t = nc.dram_tensor("idx", [128, idx_cols], I16, kind="ExternalInput")
    oh_t = nc.dram_tensor("oh", [128, blk_cols * 128], MSG_DT,
                          kind="ExternalInput")
    id_t = nc.dram_tensor("ident", [128, 128], BF16, kind="ExternalInput")
    out_t = nc.dram_tensor("out", [NPAD, D_OUT], BF16, kind="ExternalOutput")

    xT, w, dis, biasf, idx, oh, ident, out = (
        t.ap() for t in (xT_t, w_t, dis_t, bias_t, idx_t, oh_t, id_t, out_t))

    # column offsets: idx / oh blocks laid out A-groups then B-groups
    gcolA = np.zeros(NG, np.int64)     # idx col offset of A-group g
    gcolB = np.zeros(NG, np.int64)
    bcol_t = np.zeros((T, 2), np.int64)  # oh block offset of (tile, half)
    ic = bc = 0
    for gi in range(NG):
        gcolA[gi] = ic
        for t in range(gi * G, min((gi + 1) * G, T)):
            bcol_t[t, 0] = bc
            bc += int(B_A[t])
        ic += 8 * int(GA[gi])
    for gi in range(NG):
        gcolB[gi] = ic
        for t in range(gi * G, min((gi + 1) * G, T)):
            bcol_t[t, 1] = bc
            bc += int(B_B[t])
        ic += 8 * int(GB[gi])

    maxGA = int(max(GA.max(), GB.max()))

    with tile.TileContext(nc) as tc:
        with tc.tile_pool(name="const", bufs=1) as constp, \
             tc.tile_pool(name="xk", bufs=2) as xkp, \
             tc.tile_pool(name="hq", bufs=3) as hqp, \
             tc.tile_pool(name="gath", bufs=2) as gp, \
             tc.tile_pool(name="ohp", bufs=2) as ohp, \
             tc.tile_pool(name="epi", bufs=3) as epip, \
             tc.tile_pool(name="psum", bufs=4, space="PSUM") as psp, \
             tc.tile_pool(name="dram", bufs=1, space="DRAM") as dramp:

            # resident constants
            w_sb = constp.tile([128, KT, D_OUT], BF16)
            for k in range(KT):
                nc.sync.dma_start(out=w_sb[:, k, :], in_=w[k * 128:(k + 1) * 128, :])
            dis_sb = constp.tile([128, T], F32)
            nc.sync.dma_start(out=dis_sb[:], in_=dis[:])
            bias_sb = constp.tile([128, D_OUT], F32)
            nc.sync.dma_start(out=bias_sb[:], in_=biasf[:])
            idx_sb = constp.tile([128, idx_cols], I16)
            nc.sync.dma_start(out=idx_sb[:], in_=idx[:])
            ident_sb = constp.tile([128, 128], BF16)
            nc.sync.dma_start(out=ident_sb[:], in_=ident[:])
            partial = constp.tile([128, T, D_OUT], BF16)
            smbuf = constp.tile([128, T], F32)

            h_locA = dramp.tile([LOC_A, D_OUT], MSG_DT)
            h_locB = dramp.tile([LOC_B, D_OUT], MSG_DT)
            h_A = dramp.tile([ROWS_A, D_OUT], MSG_DT, addr_space="Shared")
            h_B = dramp.tile([ROWS_B, D_OUT], MSG_DT, addr_space="Shared")

            # ---- phase 1: h' = dis_src * (x @ W); partial init; 2 AGs ----
            for s in range(SUP):
                xk = xkp.tile([128, KT, SW], BF16, name="xk")
                for k in range(KT):
                    nc.sync.dma_start(
                        out=xk[:, k, :],
                        in_=xT[k * 128:(k + 1) * 128, s * SW:(s + 1) * SW])
                for t in range(SW // 128):
                    ph = psp.tile([128, D_OUT], F32, name="ph")
                    for k in range(KT):
                        nc.tensor.matmul(
                            ph[:], xk[:, k, t * 128:(t + 1) * 128],
                            w_sb[:, k, :], start=(k == 0), stop=(k == KT - 1))
                    gt = s * (SW // 128) + t
                    # partial <- h' tile (self-loop message, bf16)
                    nc.vector.tensor_scalar(
                        partial[:, gt, :], ph[:], dis_sb[:, gt:gt + 1], None,
                        mybir.AluOpType.mult)
                    if not bias_zero:
                        inv = epip.tile([128, 1], F32, name="inv", tag="inv")
                        nc.vector.reciprocal(inv[:], dis_sb[:, gt:gt + 1])
                        tmp = epip.tile([128, D_OUT], F32, name="tmp", tag="tmp")
                        nc.vector.tensor_scalar(
                            tmp[:], bias_sb[:], inv[:, 0:1], None,
                            mybir.AluOpType.mult)
                        nc.vector.tensor_tensor(
                            partial[:, gt, :], partial[:, gt, :], tmp[:],
                            mybir.AluOpType.add)
                    # fp8 copy for the allgather (scalar engine, idle here)
                    hq = hqp.tile([128, D_OUT], MSG_DT, name="hq")
                    nc.scalar.activation(hq[:], partial[:, gt, :],
                                         mybir.ActivationFunctionType.Copy)
                    r0 = gt * 128
                    if r0 < LOC_A:
                        nc.sync.dma_start(out=h_locA[r0:r0 + 128, :], in_=hq[:])
                    else:
                        nc.sync.dma_start(
                            out=h_locB[r0 - LOC_A:r0 - LOC_A + 128, :], in_=hq[:])
                if s == SUP_A - 1:
                    nc.gpsimd.collective_compute(
                        "AllGather", mybir.AluOpType.bypass,
                        replica_groups=[list(range(C))],
                        ins=[h_locA.opt()], outs=[h_A.opt()])
            nc.gpsimd.collective_compute(
                "AllGather", mybir.AluOpType.bypass,
                replica_groups=[list(range(C))],
                ins=[h_locB.opt()], outs=[h_B.opt()])

            # ---- phase 2a: A-half gathers + matmuls accumulate into partial
            for gi in range(NG):
                nga = int(GA[gi])
                ga = gp.tile([128, maxGA, D_OUT], MSG_DT, name="ga")
                icol = int(gcolA[gi])
                nc.gpsimd.dma_gather(
                    out_ap=ga[:, :nga, :], in_ap=h_A[:],
                    idxs_ap=idx_sb[:, icol:icol + 8 * nga],
                    num_idxs=nga * 128, num_idxs_reg=nga * 128,
                    elem_size=D_OUT, single_packet=(nga * 128 <= 1024))
                oh_g = ohp.tile([128, maxGA * 128], MSG_DT, name="oh_g")
                b0 = int(bcol_t[gi * G, 0])
                nc.sync.dma_start(out=oh_g[:, :nga * 128],
                                  in_=oh[:, b0 * 128:(b0 + nga) * 128])
                boff = 0
                for t in range(gi * G, min((gi + 1) * G, T)):
                    ba = int(B_A[t])
                    if ba == 0:
                        continue
                    pa = psp.tile([128, D_OUT], F32, name="pa", tag="ph")
                    for b in range(ba):
                        nc.tensor.matmul(
                            pa[:], oh_g[:, (boff + b) * 128:(boff + b + 1) * 128],
                            ga[:, boff + b, :], start=(b == 0), stop=(b == ba - 1))
                    boff += ba
                    nc.vector.tensor_tensor(partial[:, t, :], partial[:, t, :],
                                            pa[:], mybir.AluOpType.add)

            # ---- phase 2b: B-half gathers + matmuls + epilogue ----
            for gi in range(NG):
                ngb = int(GB[gi])
                gb = gp.tile([128, maxGA, D_OUT], MSG_DT, name="gb", tag="ga")
                icol = int(gcolB[gi])
                nc.gpsimd.dma_gather(
                    out_ap=gb[:, :ngb, :], in_ap=h_B[:],
                    idxs_ap=idx_sb[:, icol:icol + 8 * ngb],
                    num_idxs=ngb * 128, num_idxs_reg=ngb * 128,
                    elem_size=D_OUT, single_packet=(ngb * 128 <= 1024))
                oh_g = ohp.tile([128, maxGA * 128], MSG_DT, name="oh_gb", tag="oh_g")
                b0 = int(bcol_t[gi * G, 1])
                nc.sync.dma_start(out=oh_g[:, :ngb * 128],
                                  in_=oh[:, b0 * 128:(b0 + ngb) * 128])
                boff = 0
                for t in range(gi * G, min((gi + 1) * G, T)):
                    bb = int(B_B[t])
                    acc = psp.tile([128, D_OUT], F32, name="acc")
                    if USE_IDMM:
                        # load partial into PSUM via identity matmul, then
                        # accumulate the B-half one-hot matmuls on top
                        nc.tensor.matmul(acc[:], ident_sb[:], partial[:, t, :],
                                         start=True, stop=(bb == 0))
                        for b in range(bb):
                            nc.tensor.matmul(
                                acc[:],
                                oh_g[:, (boff + b) * 128:(boff + b + 1) * 128],
                                gb[:, boff + b, :], start=False,
                                stop=(b == bb - 1))
                    else:
                        for b in range(bb):
                            nc.tensor.matmul(
                                acc[:],
                                oh_g[:, (boff + b) * 128:(boff + b + 1) * 128],
                                gb[:, boff + b, :], start=(b == 0),
                                stop=(b == bb - 1))
                        nc.vector.tensor_tensor(acc[:], acc[:],
                                                partial[:, t, :],
                                                mybir.AluOpType.add)
                    boff += bb

                    # z = dis_dst * acc (bias folded via partial); overwrite
                    # the dead partial slice with z (bf16)
                    nc.vector.tensor_scalar(partial[:, t, :], acc[:],
                                            dis_sb[:, t:t + 1], None,
                                            mybir.AluOpType.mult)
                    # Exp + per-row sum; Ln batched after the loop (one act
                    # table load instead of two per tile)
                    ex = epip.tile([128, D_OUT], BF16, name="ex")
                    nc.scalar.activation(ex[:], partial[:, t, :],
                                         mybir.ActivationFunctionType.Exp,
                                         accum_out=smbuf[:, t:t + 1])

            # ---- batched log-sum-exp + output sweep ----
            lse = epip.tile([128, T], F32, name="lse", tag="lse")
            nc.scalar.activation(lse[:], smbuf[:],
                                 mybir.ActivationFunctionType.Ln)
            nlse = epip.tile([128, T], F32, name="nlse", tag="nlse")
            nc.vector.tensor_scalar(nlse[:], lse[:], -1.0, None,
                                    mybir.AluOpType.mult)
            for t in range(T):
                res = epip.tile([128, D_OUT], BF16, name="res")
                nc.vector.tensor_scalar(res[:], partial[:, t, :],
                                        nlse[:, t:t + 1], None,
                                        mybir.AluOpType.add)
                nc.sync.dma_start(out=out[t * 128:(t + 1) * 128, :], in_=res[:])

    nc.compile()
    return nc


def kernel(x, edge_index, weight, bias):
    global LAST_RESULTS
    x = np.asarray(x, dtype=np.float32)
    weight = np.asarray(weight, dtype=np.float32)
    bias = np.asarray(bias, dtype=np.float32)

    pp = _preprocess(x, edge_index, weight, bias)
    idx_cols = pp["idx"].shape[2]
    blk_cols = pp["oh"].shape[2] // 128
    nc = _build(pp["B_A"], pp["B_B"], pp["GA"], pp["GB"], idx_cols, blk_cols,
                pp["bias_zero"])

    in_maps = []
    for c in range(C):
        in_maps.append({
            "xT": np.ascontiguousarray(pp["xT"][c]),
            "w": pp["w"],
            "dis": np.ascontiguousarray(pp["dis"][c]),
            "biasf": pp["bias"],
            "idx": np.ascontiguousarray(pp["idx"][c]),
            "oh": np.ascontiguousarray(pp["oh"][c]),
            "ident": pp["ident"],
        })

    res = run_bass_kernel_spmd(nc, in_maps, core_ids=list(range(C)))
    LAST_RESULTS = res

    out = np.empty((N_NODES, D_OUT), np.float32)
    for c in range(C):
        out[c * NLOC:(c + 1) * NLOC] = res.results[c]["out"][:NLOC].astype(np.float32)
    return out
